# revision 5
# baseline (speedup 1.0000x reference)
"""Trainium2 Bass kernel for nn_CCSequenceModel (2-layer GRU encoder +
autoregressive 2-layer GRU decoder with gated CV head).

Sharding: pure data parallel. B=16384 is split across 8 cores (2048 each).
Per core the recurrent state lives transposed [hidden, batch] and "folded":
batch half A (1024 elems) on partitions 0-63, half B on partitions 64-127.

v3:
- 64-wide contractions (hh, e1/d1-ih) run as quadrant PAIRS (half A in array
  quadrant (0,0), half B in (64,64)); the pair's second LDWEIGHTS and MATMUL
  hide under the first, so a pair costs ~one MM. Thin inputs (e0-ih K=12,
  d0-ih K=2, heads) use block-diagonal [K,128] weights (one MM, tiny LDW).
- All elementwise ops run FULL-WIDTH [128, 1024] (one op per cell instead of
  two): psum tiles are [128, 1024] two-bank tiles, matmuls write bank-sized
  slices. Halves the fixed ~250ns/op engine overhead count.
- Blend restructured as h' = n*(1-z) + z*h: the z*h product has no
  dependence on tanh, so it runs on the otherwise-idle GPSIMD engine OFF the
  critical path; the tail after tanh is two cheap DVE ops.
- Optional dummy matmuls (KDUM env) fill PE idle between cells to keep the
  HAM clock-gate warm (cold PE runs at 1.2 GHz instead of 2.4).
"""

import os
import sys

sys.path.insert(0, "/opt/trn_rl_repo")

import numpy as np
import ml_dtypes

import concourse.bass as bass
import concourse.bacc as bacc_mod
import concourse.mybir as mybir
import concourse.tile as tile
from concourse.bass_utils import run_bass_kernel_spmd

F32 = mybir.dt.float32
BF16 = mybir.dt.bfloat16
AF = mybir.ActivationFunctionType
OP = mybir.AluOpType

H = 64
NCORES = 8
BLOC = 2048          # batch per core
NW = 1024            # folded free width (batch half)
MM_N = 512           # moving free dim per matmul instruction (1 psum bank)
BF16_T = ml_dtypes.bfloat16

GPSZH = int(os.environ.get("KGPSZH", "1"))   # z*h on GPSIMD
KDUM = int(os.environ.get("KDUM", "16"))     # HAM-warming dummy matmuls/cell

CELLS = ["e0", "e1", "d0", "d1"]

# ih width per cell: "pair" = 64-col quadrant blocks, else block-diag rows
IH_KIND = {"e0": 12, "e1": "pair", "d0": 2, "d1": "pair"}


def _wcols():
    cols = {}
    cur = 0
    for c in CELLS:
        for g in ["r", "z", "n"]:
            w = 64 if IH_KIND[c] == "pair" else 128
            cols[c + "ih_" + g] = cur
            cur += w
            cols[c + "hh_" + g] = cur
            cur += 64
    cols["hd"] = cur  # heads: on_A, on_B, cv_A, cv_B
    cur += 4
    return cols, cur


def _bcols():
    cols = {}
    cur = 0
    for c in CELLS:
        for p in ["sr", "sz", "bin", "bhn"]:
            cols[c + p] = cur
            cur += 1
    cols["bon"] = cur
    cur += 1
    cols["bcv"] = cur
    cur += 1
    return cols, cur


WCOLS, NWCOL = _wcols()
BCOLS, NBCOL = _bcols()


def pack_weights(iv):
    """Returns (wpack [128, NWCOL] bf16, bpack [128, NBCOL] f32)."""
    wp = np.zeros((128, NWCOL), np.float32)
    bp = np.zeros((128, NBCOL), np.float32)

    def put_pair(col, wt):  # wt [64, 64], duplicated on both partition halves
        wp[0:64, col : col + 64] = wt
        wp[64:128, col : col + 64] = wt

    def put_diag(col, wt, ka):  # wt [ka, 64] -> [2*ka, 128] block-diagonal
        wp[0:ka, col : col + 64] = wt
        wp[ka : 2 * ka, col + 64 : col + 128] = wt

    def put_b(col, v):
        v = np.asarray(v, np.float32).reshape(-1)
        bp[0 : v.size, col] = v
        bp[64 : 64 + v.size, col] = v

    cfg = {
        "e0": ("e_Wih0", "e_Whh0", "e_bih0", "e_bhh0"),
        "e1": ("e_Wih1", "e_Whh1", "e_bih1", "e_bhh1"),
        "d0": ("d_Wih0", "d_Whh0", "d_bih0", "d_bhh0"),
        "d1": ("d_Wih1", "d_Whh1", "d_bih1", "d_bhh1"),
    }
    for c, (wih, whh, bih, bhh) in cfg.items():
        Wih, Whh = np.asarray(iv[wih], np.float32), np.asarray(iv[whh], np.float32)
        bih, bhh = np.asarray(iv[bih], np.float32), np.asarray(iv[bhh], np.float32)
        kin = Wih.shape[1]
        for gi, g in enumerate(["r", "z", "n"]):
            wt_ih = Wih[gi * 64 : (gi + 1) * 64, :].T
            wt_hh = Whh[gi * 64 : (gi + 1) * 64, :].T
            if IH_KIND[c] == "pair":
                put_pair(WCOLS[c + "ih_" + g], wt_ih)
            else:
                put_diag(WCOLS[c + "ih_" + g], wt_ih, kin)
            put_pair(WCOLS[c + "hh_" + g], wt_hh)
        put_b(BCOLS[c + "sr"], bih[0:64] + bhh[0:64])
        put_b(BCOLS[c + "sz"], bih[64:128] + bhh[64:128])
        put_b(BCOLS[c + "bin"], bih[128:192])
        put_b(BCOLS[c + "bhn"], bhh[128:192])
    hd = WCOLS["hd"]
    won = np.asarray(iv["W_on"], np.float32).reshape(64)
    wcv = np.asarray(iv["W_cv"], np.float32).reshape(64)
    wp[0:64, hd + 0] = won
    wp[64:128, hd + 1] = won
    wp[0:64, hd + 2] = wcv
    wp[64:128, hd + 3] = wcv
    bp[0:2, BCOLS["bon"]] = float(np.asarray(iv["b_on"]).reshape(()))
    bp[0:2, BCOLS["bcv"]] = float(np.asarray(iv["b_cv"]).reshape(()))
    return wp.astype(BF16_T), bp


def build_program(T, DEC):
    """Emit the per-core Bass program (same program on all 8 cores)."""
    nc = bacc_mod.Bacc(None, target_bir_lowering=False)
    xpack = nc.declare_dram_parameter("xpack", [T, 12, NW], BF16, isOutput=False)
    wpack = nc.declare_dram_parameter("wpack", [128, NWCOL], BF16, isOutput=False)
    bpack = nc.declare_dram_parameter("bpack", [128, NBCOL], F32, isOutput=False)
    outd = nc.declare_dram_parameter("out", [DEC, 2, NW], BF16, isOutput=True)

    CHUNKS = [slice(c0, c0 + MM_N) for c0 in range(0, NW, MM_N)]

    with tile.TileContext(nc) as tc:
        with (
            tc.tile_pool(name="const", bufs=1) as const,
            tc.tile_pool(name="state", bufs=1) as state,
            tc.tile_pool(name="xin", bufs=4) as xin,
            tc.tile_pool(name="tmp", bufs=2) as tmp,
            tc.tile_pool(name="psum", bufs=1, space="PSUM") as psum,
        ):
            wsb = const.tile([128, NWCOL], BF16)
            bsb = const.tile([128, NBCOL], F32)
            nc.sync.dma_start(out=wsb[:, :], in_=wpack[:, :])
            nc.sync.dma_start(out=bsb[:, :], in_=bpack[:, :])

            S0 = state.tile([128, NW], BF16, tag="S0")
            S1 = state.tile([128, NW], BF16, tag="S1")
            PV = state.tile([2, NW], BF16, tag="PV")  # decoder prev [A; B]
            nc.vector.memset(S0[:, :], 0.0)
            nc.vector.memset(S1[:, :], 0.0)
            nc.vector.memset(PV[:, :], 0.0)

            def B_(name, p=128):
                c = BCOLS[name]
                return bsb[0:p, c : c + 1]

            def qpair(P, wname, hS, sl, start, stop):
                """Quadrant pair: half A in (0,0), half B in (64,64)."""
                c = WCOLS[wname]
                for r0 in (0, 64):
                    nc.tensor.matmul(
                        P[r0 : r0 + 64, sl],
                        wsb[r0 : r0 + 64, c : c + 64],
                        hS[r0 : r0 + 64, sl],
                        start=start, stop=stop, skip_group_check=True,
                    )

            def bdiag(P, wname, IN, k, sl, start, stop):
                """Block-diagonal thin-input matmul: one MM for both halves."""
                c = WCOLS[wname]
                nc.tensor.matmul(
                    P[:, sl], wsb[0:k, c : c + 128], IN[0:k, sl],
                    start=start, stop=stop, skip_group_check=True,
                )

            def dummies(k):
                """HAM-warming junk matmuls into P_d (released psum)."""
                for _ in range(k):
                    Pd = psum.tile([128, NW], F32, tag="P_r", name="Pd")
                    nc.tensor.matmul(
                        Pd[0:64, 0:64], wsb[0:64, 0:64], wsb[0:64, 0:64],
                        start=True, stop=True, skip_group_check=True,
                    )

            def gru_cell(cell, IN, hS, outS, ih_first=False):
                """IN: input tile ([12|2|128] rows used); hS/outS: state tile
                (updated in place, full width)."""
                kind = IH_KIND[cell]
                P_r = psum.tile([128, NW], F32, tag="P_r", name="P_r")
                P_z = psum.tile([128, NW], F32, tag="P_z", name="P_z")
                P_i = psum.tile([128, NW], F32, tag="P_i", name="P_i")
                P_h = psum.tile([128, NW], F32, tag="P_h", name="P_h")

                def ih_phase(first):
                    last = not first
                    if kind == "pair":
                        for sl in CHUNKS:
                            qpair(P_r, cell + "ih_r", IN, sl, first, last)
                        for sl in CHUNKS:
                            qpair(P_i, cell + "ih_n", IN, sl, True, True)
                        for sl in CHUNKS:
                            qpair(P_z, cell + "ih_z", IN, sl, first, last)
                    else:
                        for sl in CHUNKS:
                            bdiag(P_r, cell + "ih_r", IN, kind, sl, first, last)
                        for sl in CHUNKS:
                            bdiag(P_i, cell + "ih_n", IN, kind, sl, True, True)
                        for sl in CHUNKS:
                            bdiag(P_z, cell + "ih_z", IN, kind, sl, first, last)

                def hh_phase(first):
                    last = not first
                    for sl in CHUNKS:
                        qpair(P_r, cell + "hh_r", hS, sl, first, last)
                    for sl in CHUNKS:
                        qpair(P_h, cell + "hh_n", hS, sl, True, True)
                    for sl in CHUNKS:
                        qpair(P_z, cell + "hh_z", hS, sl, first, last)

                if ih_first:
                    ih_phase(True)
                    hh_phase(False)
                else:
                    hh_phase(True)
                    ih_phase(False)

                R = tmp.tile([128, NW], BF16, tag="R", name="R")
                Z = tmp.tile([128, NW], BF16, tag="Z", name="Z")
                A_ = tmp.tile([128, NW], BF16, tag="A", name="A")
                Sm = tmp.tile([128, NW], BF16, tag="Sm", name="Sm")
                N_ = tmp.tile([128, NW], BF16, tag="N", name="N")
                W1 = tmp.tile([128, NW], BF16, tag="W1", name="W1")
                ZH = tmp.tile([128, NW], BF16, tag="ZH", name="ZH")
                G = tmp.tile([128, NW], BF16, tag="G", name="G")

                nc.scalar.activation(R[:, :], P_r[:, :], AF.Sigmoid, bias=B_(cell + "sr"))
                nc.scalar.activation(Z[:, :], P_z[:, :], AF.Sigmoid, bias=B_(cell + "sz"))
                # A = (hn + bhn) * r ; Sm = (inn + bin) + A ; n = tanh(Sm)
                nc.vector.scalar_tensor_tensor(
                    out=A_[:, :], in0=P_h[:, :], scalar=B_(cell + "bhn"),
                    in1=R[:, :], op0=OP.add, op1=OP.mult,
                )
                nc.vector.scalar_tensor_tensor(
                    out=Sm[:, :], in0=P_i[:, :], scalar=B_(cell + "bin"),
                    in1=A_[:, :], op0=OP.add, op1=OP.add,
                )
                nc.scalar.activation(N_[:, :], Sm[:, :], AF.Tanh)
                # h' = n*(1-z) + z*h: ZH = z*h runs on GPSIMD off the tanh
                # critical path; W1 = 1-z is a cheap 4x-mode tensor_scalar.
                zh_eng = nc.gpsimd if GPSZH else nc.vector
                zh_eng.tensor_tensor(
                    out=ZH[:, :], in0=Z[:, :], in1=outS[:, :], op=OP.mult
                )
                nc.vector.tensor_scalar(
                    out=W1[:, :], in0=Z[:, :], scalar1=-1.0, scalar2=1.0,
                    op0=OP.mult, op1=OP.add,
                )
                nc.vector.tensor_tensor(out=G[:, :], in0=N_[:, :], in1=W1[:, :], op=OP.mult)
                nc.vector.tensor_tensor(out=outS[:, :], in0=G[:, :], in1=ZH[:, :], op=OP.add)

            # ---------------- encoder ----------------
            for t in range(T):
                xst = xin.tile([12, NW], BF16, tag="xst")
                nc.sync.dma_start(out=xst[:, :], in_=xpack[t])
                gru_cell("e0", xst, S0, S0, ih_first=True)
                dummies(KDUM)
                gru_cell("e1", S0, S1, S1)
                dummies(KDUM)

            # ---------------- decoder ----------------
            hd = WCOLS["hd"]
            for t in range(DEC):
                gru_cell("d0", PV, S0, S0)
                dummies(KDUM)
                gru_cell("d1", S0, S1, S1)
                dummies(KDUM)
                P_on = psum.tile([128, NW], F32, tag="P_i", name="P_on")
                P_cv = psum.tile([128, NW], F32, tag="P_h", name="P_cv")
                for sl in CHUNKS:
                    nc.tensor.matmul(
                        P_on[0:2, sl], wsb[:, hd : hd + 2], S1[:, sl],
                        start=True, stop=True, skip_group_check=True,
                    )
                    nc.tensor.matmul(
                        P_cv[0:2, sl], wsb[:, hd + 2 : hd + 4], S1[:, sl],
                        start=True, stop=True, skip_group_check=True,
                    )
                MK = tmp.tile([2, NW], BF16, tag="MK", name="MK")
                # mask = (logit + b_on) > 0 ; prev = (cv + b_cv) * mask
                nc.vector.tensor_scalar(
                    out=MK[:, :], in0=P_on[0:2, :],
                    scalar1=B_("bon", 2), scalar2=0.0,
                    op0=OP.add, op1=OP.is_gt,
                )
                nc.vector.scalar_tensor_tensor(
                    out=PV[:, :], in0=P_cv[0:2, :], scalar=B_("bcv", 2),
                    in1=MK[:, :], op0=OP.add, op1=OP.mult,
                )
                nc.sync.dma_start(out=outd[t], in_=PV[:, :])
    nc.compile()
    return nc


_CACHE = {}


def get_program(T, DEC):
    key = (T, DEC, MM_N, GPSZH, KDUM)
    if key not in _CACHE:
        _CACHE[key] = build_program(T, DEC)
    return _CACHE[key]


def pack_x(x):
    """x [B, T, NI] f32 -> per-core list of xpack [T, 12, NW] bf16."""
    B, T, NI = x.shape
    out = []
    for c in range(NCORES):
        xs = x[c * BLOC : (c + 1) * BLOC]  # [2048, T, 6]
        xp = np.ascontiguousarray(
            xs.reshape(2, NW, T, NI).transpose(2, 0, 3, 1).reshape(T, 12, NW)
        )
        out.append(xp.astype(BF16_T))
    return out


def run(x, target_len, weights, trace=False, trace_kwargs=None):
    T = x.shape[1]
    DEC = int(target_len)
    nc = get_program(T, DEC)
    wp, bp = pack_weights(weights)
    xps = pack_x(np.asarray(x, np.float32))
    in_maps = [{"xpack": xps[c], "wpack": wp, "bpack": bp} for c in range(NCORES)]
    res = run_bass_kernel_spmd(
        nc, in_maps, list(range(NCORES)), trace=trace, **(trace_kwargs or {})
    )
    outs = [np.asarray(res.results[c]["out"], np.float32) for c in range(NCORES)]
    # [DEC, 2, NW] per core -> [B, DEC, 1]
    full = np.concatenate(
        [o.transpose(1, 2, 0).reshape(BLOC, DEC, 1) for o in outs], axis=0
    )
    return full, res


def kernel(**inputs):
    x = np.asarray(inputs["x"], np.float32)
    target_len = int(np.asarray(inputs["target_len"]).reshape(()))
    weights = {k: v for k, v in inputs.items() if k not in ("x", "target_len")}
    full, _ = run(x, target_len, weights)
    return full.astype(np.float32)


if __name__ == "__main__":
    rng = np.random.default_rng(0)
    B, T, NI, DEC = 16384, 4, 6, 3
    iv = {
        "x": rng.standard_normal((B, T, NI), dtype=np.float32),
        "target_len": DEC,
    }
    s = 1.0 / np.sqrt(H)
    for nm, shp in [
        ("e_Wih0", (192, 6)), ("e_Whh0", (192, 64)), ("e_bih0", (192,)), ("e_bhh0", (192,)),
        ("e_Wih1", (192, 64)), ("e_Whh1", (192, 64)), ("e_bih1", (192,)), ("e_bhh1", (192,)),
        ("d_Wih0", (192, 1)), ("d_Whh0", (192, 64)), ("d_bih0", (192,)), ("d_bhh0", (192,)),
        ("d_Wih1", (192, 64)), ("d_Whh1", (192, 64)), ("d_bih1", (192,)), ("d_bhh1", (192,)),
        ("W_on", (1, 64)), ("b_on", (1,)), ("W_cv", (1, 64)), ("b_cv", (1,)),
    ]:
        iv[nm] = (rng.uniform(-s, s, shp)).astype(np.float32)
    out = kernel(**iv)
    print("kernel out", out.shape, out.dtype, float(np.abs(out).mean()))


# revision 6
# speedup vs baseline: 1.4482x; 1.4482x over previous
"""Trainium2 Bass kernel for nn_CCSequenceModel (2-layer GRU encoder +
autoregressive 2-layer GRU decoder with gated CV head).

Sharding: pure data parallel. B=16384 is split across 8 cores (2048 each).
Per core the recurrent state lives transposed [hidden, batch] and "folded":
batch half A (1024 elems) on partitions 0-63, half B on partitions 64-127.

v4 (chunk-pipelined):
- 64-wide contractions run as quadrant PAIRS (half A in array quadrant
  (0,0), half B in (64,64)); the pair's second LDWEIGHTS/MATMUL hide under
  the first. Thin inputs (e0-ih K=12, d0-ih K=2, heads) use block-diagonal
  [K,128] weights: one MM for both halves, tiny LDW.
- All elementwise work is chunked at [128, 512] and phase-ordered so each
  engine queue never stalls behind a not-yet-ready op, and the state update
  lands per chunk (the next cell's input matmuls start on chunk 0 while
  chunk 1 is still in flight).
- Blend restructured as h' = n*(1-z) + z*h: z*h runs on the otherwise-idle
  GPSIMD engine OFF the tanh critical path; the post-tanh tail is two cheap
  DVE ops.
- Decoder heads: one [128,4] block-diag weight -> [2,512] psum slices, two
  small DVE ops write the compact [2, NW] prev tile that feeds d0 directly.
- Optional dummy matmuls (KDUM) fill PE idle between cells to keep the HAM
  clock-gate warm (a cold PE streams at 1.2 GHz instead of 2.4).
"""

import os
import sys

sys.path.insert(0, "/opt/trn_rl_repo")

import numpy as np
import ml_dtypes

import concourse.bass as bass
import concourse.bacc as bacc_mod
import concourse.mybir as mybir
import concourse.tile as tile
from concourse.bass_utils import run_bass_kernel_spmd

F32 = mybir.dt.float32
BF16 = mybir.dt.bfloat16
AF = mybir.ActivationFunctionType
OP = mybir.AluOpType

H = 64
NCORES = 8
BLOC = 2048          # batch per core
NW = 1024            # folded free width (batch half)
MM_N = 512           # moving free dim per matmul instruction (1 psum bank)
BF16_T = ml_dtypes.bfloat16

GPSZH = int(os.environ.get("KGPSZH", "1"))   # z*h on GPSIMD
KDUM = int(os.environ.get("KDUM", "12"))     # HAM-warming dummy matmuls/cell

CELLS = ["e0", "e1", "d0", "d1"]
IH_KIND = {"e0": 12, "e1": "pair", "d0": 2, "d1": "pair"}


def _wcols():
    cols = {}
    cur = 0
    for c in CELLS:
        for g in ["r", "z", "n"]:
            w = 64 if IH_KIND[c] == "pair" else 128
            cols[c + "ih_" + g] = cur
            cur += w
            cols[c + "hh_" + g] = cur
            cur += 64
    cols["hd"] = cur  # heads: on_A, on_B, cv_A, cv_B
    cur += 4
    return cols, cur


def _bcols():
    cols = {}
    cur = 0
    for c in CELLS:
        for p in ["sr", "sz", "bin", "bhn"]:
            cols[c + p] = cur
            cur += 1
    cols["bon"] = cur
    cur += 1
    cols["bcv"] = cur
    cur += 1
    return cols, cur


WCOLS, NWCOL = _wcols()
BCOLS, NBCOL = _bcols()


def pack_weights(iv):
    """Returns (wpack [128, NWCOL] bf16, bpack [128, NBCOL] f32)."""
    wp = np.zeros((128, NWCOL), np.float32)
    bp = np.zeros((128, NBCOL), np.float32)

    def put_pair(col, wt):  # wt [64, 64], duplicated on both partition halves
        wp[0:64, col : col + 64] = wt
        wp[64:128, col : col + 64] = wt

    def put_diag(col, wt, ka):  # wt [ka, 64] -> [2*ka, 128] block-diagonal
        wp[0:ka, col : col + 64] = wt
        wp[ka : 2 * ka, col + 64 : col + 128] = wt

    def put_b(col, v):
        v = np.asarray(v, np.float32).reshape(-1)
        bp[0 : v.size, col] = v
        bp[64 : 64 + v.size, col] = v

    cfg = {
        "e0": ("e_Wih0", "e_Whh0", "e_bih0", "e_bhh0"),
        "e1": ("e_Wih1", "e_Whh1", "e_bih1", "e_bhh1"),
        "d0": ("d_Wih0", "d_Whh0", "d_bih0", "d_bhh0"),
        "d1": ("d_Wih1", "d_Whh1", "d_bih1", "d_bhh1"),
    }
    for c, (wih, whh, bih, bhh) in cfg.items():
        Wih, Whh = np.asarray(iv[wih], np.float32), np.asarray(iv[whh], np.float32)
        bih, bhh = np.asarray(iv[bih], np.float32), np.asarray(iv[bhh], np.float32)
        kin = Wih.shape[1]
        for gi, g in enumerate(["r", "z", "n"]):
            wt_ih = Wih[gi * 64 : (gi + 1) * 64, :].T
            wt_hh = Whh[gi * 64 : (gi + 1) * 64, :].T
            if IH_KIND[c] == "pair":
                put_pair(WCOLS[c + "ih_" + g], wt_ih)
            else:
                put_diag(WCOLS[c + "ih_" + g], wt_ih, kin)
            put_pair(WCOLS[c + "hh_" + g], wt_hh)
        put_b(BCOLS[c + "sr"], bih[0:64] + bhh[0:64])
        put_b(BCOLS[c + "sz"], bih[64:128] + bhh[64:128])
        put_b(BCOLS[c + "bin"], bih[128:192])
        put_b(BCOLS[c + "bhn"], bhh[128:192])
    hd = WCOLS["hd"]
    won = np.asarray(iv["W_on"], np.float32).reshape(64)
    wcv = np.asarray(iv["W_cv"], np.float32).reshape(64)
    wp[0:64, hd + 0] = won
    wp[64:128, hd + 1] = won
    wp[0:64, hd + 2] = wcv
    wp[64:128, hd + 3] = wcv
    bp[0:2, BCOLS["bon"]] = float(np.asarray(iv["b_on"]).reshape(()))
    bp[0:2, BCOLS["bcv"]] = float(np.asarray(iv["b_cv"]).reshape(()))
    return wp.astype(BF16_T), bp


def build_program(T, DEC):
    """Emit the per-core Bass program (same program on all 8 cores)."""
    nc = bacc_mod.Bacc(None, target_bir_lowering=False)
    xpack = nc.declare_dram_parameter("xpack", [T, 12, NW], BF16, isOutput=False)
    wpack = nc.declare_dram_parameter("wpack", [128, NWCOL], BF16, isOutput=False)
    bpack = nc.declare_dram_parameter("bpack", [128, NBCOL], F32, isOutput=False)
    outd = nc.declare_dram_parameter("out", [DEC, 2, NW], BF16, isOutput=True)

    CHUNKS = [slice(c0, c0 + MM_N) for c0 in range(0, NW, MM_N)]
    NCH = len(CHUNKS)

    with tile.TileContext(nc) as tc:
        with (
            tc.tile_pool(name="const", bufs=1) as const,
            tc.tile_pool(name="state", bufs=1) as state,
            tc.tile_pool(name="xin", bufs=4) as xin,
            tc.tile_pool(name="tmp", bufs=3) as tmp,
            tc.tile_pool(name="psum", bufs=2, space="PSUM") as psum,
        ):
            wsb = const.tile([128, NWCOL], BF16)
            bsb = const.tile([128, NBCOL], F32)
            nc.sync.dma_start(out=wsb[:, :], in_=wpack[:, :])
            nc.sync.dma_start(out=bsb[:, :], in_=bpack[:, :])

            S0 = state.tile([128, NW], BF16, tag="S0")
            S1 = state.tile([128, NW], BF16, tag="S1")
            PV = state.tile([2, NW], BF16, tag="PV")  # decoder prev [A; B]
            nc.vector.memset(S0[:, :], 0.0)
            nc.vector.memset(S1[:, :], 0.0)
            nc.vector.memset(PV[:, :], 0.0)

            def B_(name, p=128):
                c = BCOLS[name]
                return bsb[0:p, c : c + 1]

            def qpair(P, wname, hS, sl, start, stop):
                """Quadrant pair: half A in (0,0), half B in (64,64)."""
                c = WCOLS[wname]
                for r0 in (0, 64):
                    nc.tensor.matmul(
                        P[r0 : r0 + 64, :],
                        wsb[r0 : r0 + 64, c : c + 64],
                        hS[r0 : r0 + 64, sl],
                        start=start, stop=stop, skip_group_check=True,
                    )

            def bdiag(P, wname, IN, k, sl, start, stop):
                """Block-diagonal thin-input matmul: one MM for both halves."""
                c = WCOLS[wname]
                nc.tensor.matmul(
                    P[:, :], wsb[0:k, c : c + 128], IN[0:k, sl],
                    start=start, stop=stop, skip_group_check=True,
                )

            def dummies(k):
                """HAM-warming junk matmuls into a rotated psum buffer."""
                for j in range(k):
                    Pd = psum.tile([128, MM_N], F32, tag="P_r", name="Pd")
                    nc.tensor.matmul(
                        Pd[0:64, 0:64], wsb[0:64, 0:64], wsb[0:64, 0:64],
                        start=True, stop=True, skip_group_check=True,
                    )

            def gru_cell(cell, IN, hS, outS, ih_first=False):
                """IN: input tile; hS/outS: state tile, updated in place
                chunk-by-chunk."""
                kind = IH_KIND[cell]
                Ps = {}
                for i in range(NCH):
                    Ps[i] = {
                        "r": psum.tile([128, MM_N], F32, tag="P_r", name=f"P_r{i}"),
                        "z": psum.tile([128, MM_N], F32, tag="P_z", name=f"P_z{i}"),
                        "i": psum.tile([128, MM_N], F32, tag="P_i", name=f"P_i{i}"),
                        "h": psum.tile([128, MM_N], F32, tag="P_h", name=f"P_h{i}"),
                    }

                def ih_phase(first):
                    last = not first
                    for i, sl in enumerate(CHUNKS):
                        P = Ps[i]
                        if kind == "pair":
                            qpair(P["r"], cell + "ih_r", IN, sl, first, last)
                            qpair(P["i"], cell + "ih_n", IN, sl, True, True)
                            qpair(P["z"], cell + "ih_z", IN, sl, first, last)
                        else:
                            bdiag(P["r"], cell + "ih_r", IN, kind, sl, first, last)
                            bdiag(P["i"], cell + "ih_n", IN, kind, sl, True, True)
                            bdiag(P["z"], cell + "ih_z", IN, kind, sl, first, last)

                def hh_phase(first):
                    last = not first
                    for i, sl in enumerate(CHUNKS):
                        P = Ps[i]
                        qpair(P["r"], cell + "hh_r", hS, sl, first, last)
                        qpair(P["h"], cell + "hh_n", hS, sl, True, True)
                        qpair(P["z"], cell + "hh_z", hS, sl, first, last)

                if ih_first:
                    ih_phase(True)
                    hh_phase(False)
                else:
                    hh_phase(True)
                    ih_phase(False)

                R, Z, N_ = {}, {}, {}
                for i in range(NCH):
                    R[i] = tmp.tile([128, MM_N], BF16, tag="R", name=f"R{i}")
                    Z[i] = tmp.tile([128, MM_N], BF16, tag="Z", name=f"Z{i}")
                    N_[i] = tmp.tile([128, MM_N], BF16, tag="N", name=f"N{i}")
                # sigmoids first (both chunks), then the tanh-arg chain, then
                # the blend; ZH = z*h runs on GPSIMD off the critical path.
                for i in range(NCH):
                    nc.scalar.activation(
                        R[i][:, :], Ps[i]["r"][:, :], AF.Sigmoid, bias=B_(cell + "sr")
                    )
                    nc.scalar.activation(
                        Z[i][:, :], Ps[i]["z"][:, :], AF.Sigmoid, bias=B_(cell + "sz")
                    )
                ZH = {}
                zh_eng = nc.gpsimd if GPSZH else nc.vector
                for i, sl in enumerate(CHUNKS):
                    ZH[i] = tmp.tile([128, MM_N], BF16, tag="ZH", name=f"ZH{i}")
                    zh_eng.tensor_tensor(
                        out=ZH[i][:, :], in0=Z[i][:, :], in1=outS[:, sl], op=OP.mult
                    )
                Sms = {}
                for i in range(NCH):
                    A_ = tmp.tile([128, MM_N], BF16, tag="A", name=f"A{i}")
                    Sm = tmp.tile([128, MM_N], BF16, tag="Sm", name=f"Sm{i}")
                    nc.vector.scalar_tensor_tensor(
                        out=A_[:, :], in0=Ps[i]["h"][:, :], scalar=B_(cell + "bhn"),
                        in1=R[i][:, :], op0=OP.add, op1=OP.mult,
                    )
                    nc.vector.scalar_tensor_tensor(
                        out=Sm[:, :], in0=Ps[i]["i"][:, :], scalar=B_(cell + "bin"),
                        in1=A_[:, :], op0=OP.add, op1=OP.add,
                    )
                    Sms[i] = Sm
                for i in range(NCH):
                    nc.scalar.activation(N_[i][:, :], Sms[i][:, :], AF.Tanh)
                for i, sl in enumerate(CHUNKS):
                    W1 = tmp.tile([128, MM_N], BF16, tag="W1", name=f"W1{i}")
                    G = tmp.tile([128, MM_N], BF16, tag="G", name=f"G{i}")
                    nc.vector.tensor_scalar(
                        out=W1[:, :], in0=Z[i][:, :], scalar1=-1.0, scalar2=1.0,
                        op0=OP.mult, op1=OP.add,
                    )
                    nc.vector.tensor_tensor(
                        out=G[:, :], in0=N_[i][:, :], in1=W1[:, :], op=OP.mult
                    )
                    nc.vector.tensor_tensor(
                        out=outS[:, sl], in0=G[:, :], in1=ZH[i][:, :], op=OP.add
                    )

            # ---------------- encoder ----------------
            for t in range(T):
                xst = xin.tile([12, NW], BF16, tag="xst")
                nc.sync.dma_start(out=xst[:, :], in_=xpack[t])
                gru_cell("e0", xst, S0, S0, ih_first=True)
                dummies(KDUM)
                gru_cell("e1", S0, S1, S1)
                dummies(KDUM)

            # ---------------- decoder ----------------
            hd = WCOLS["hd"]
            for t in range(DEC):
                gru_cell("d0", PV, S0, S0)
                dummies(KDUM)
                gru_cell("d1", S0, S1, S1)
                dummies(KDUM)
                for i, sl in enumerate(CHUNKS):
                    P_on = psum.tile([128, MM_N], F32, tag="P_i", name=f"P_on{i}")
                    P_cv = psum.tile([128, MM_N], F32, tag="P_h", name=f"P_cv{i}")
                    nc.tensor.matmul(
                        P_on[0:2, :], wsb[:, hd : hd + 2], S1[:, sl],
                        start=True, stop=True, skip_group_check=True,
                    )
                    nc.tensor.matmul(
                        P_cv[0:2, :], wsb[:, hd + 2 : hd + 4], S1[:, sl],
                        start=True, stop=True, skip_group_check=True,
                    )
                    MK = tmp.tile([2, MM_N], BF16, tag="MK", name=f"MK{i}")
                    # mask = (logit + b_on) > 0 ; prev = (cv + b_cv) * mask
                    nc.vector.tensor_scalar(
                        out=MK[:, :], in0=P_on[0:2, :],
                        scalar1=B_("bon", 2), scalar2=0.0,
                        op0=OP.add, op1=OP.is_gt,
                    )
                    nc.vector.scalar_tensor_tensor(
                        out=PV[:, sl], in0=P_cv[0:2, :], scalar=B_("bcv", 2),
                        in1=MK[:, :], op0=OP.add, op1=OP.mult,
                    )
                nc.sync.dma_start(out=outd[t], in_=PV[:, :])
    nc.compile()
    return nc


_CACHE = {}


def get_program(T, DEC):
    key = (T, DEC, MM_N, GPSZH, KDUM)
    if key not in _CACHE:
        _CACHE[key] = build_program(T, DEC)
    return _CACHE[key]


def pack_x(x):
    """x [B, T, NI] f32 -> per-core list of xpack [T, 12, NW] bf16."""
    B, T, NI = x.shape
    out = []
    for c in range(NCORES):
        xs = x[c * BLOC : (c + 1) * BLOC]  # [2048, T, 6]
        xp = np.ascontiguousarray(
            xs.reshape(2, NW, T, NI).transpose(2, 0, 3, 1).reshape(T, 12, NW)
        )
        out.append(xp.astype(BF16_T))
    return out


def run(x, target_len, weights, trace=False, trace_kwargs=None):
    T = x.shape[1]
    DEC = int(target_len)
    nc = get_program(T, DEC)
    wp, bp = pack_weights(weights)
    xps = pack_x(np.asarray(x, np.float32))
    in_maps = [{"xpack": xps[c], "wpack": wp, "bpack": bp} for c in range(NCORES)]
    res = run_bass_kernel_spmd(
        nc, in_maps, list(range(NCORES)), trace=trace, **(trace_kwargs or {})
    )
    outs = [np.asarray(res.results[c]["out"], np.float32) for c in range(NCORES)]
    # [DEC, 2, NW] per core -> [B, DEC, 1]
    full = np.concatenate(
        [o.transpose(1, 2, 0).reshape(BLOC, DEC, 1) for o in outs], axis=0
    )
    return full, res


def kernel(**inputs):
    x = np.asarray(inputs["x"], np.float32)
    target_len = int(np.asarray(inputs["target_len"]).reshape(()))
    weights = {k: v for k, v in inputs.items() if k not in ("x", "target_len")}
    full, _ = run(x, target_len, weights)
    return full.astype(np.float32)


if __name__ == "__main__":
    rng = np.random.default_rng(0)
    B, T, NI, DEC = 16384, 4, 6, 3
    iv = {
        "x": rng.standard_normal((B, T, NI), dtype=np.float32),
        "target_len": DEC,
    }
    s = 1.0 / np.sqrt(H)
    for nm, shp in [
        ("e_Wih0", (192, 6)), ("e_Whh0", (192, 64)), ("e_bih0", (192,)), ("e_bhh0", (192,)),
        ("e_Wih1", (192, 64)), ("e_Whh1", (192, 64)), ("e_bih1", (192,)), ("e_bhh1", (192,)),
        ("d_Wih0", (192, 1)), ("d_Whh0", (192, 64)), ("d_bih0", (192,)), ("d_bhh0", (192,)),
        ("d_Wih1", (192, 64)), ("d_Whh1", (192, 64)), ("d_bih1", (192,)), ("d_bhh1", (192,)),
        ("W_on", (1, 64)), ("b_on", (1,)), ("W_cv", (1, 64)), ("b_cv", (1,)),
    ]:
        iv[nm] = (rng.uniform(-s, s, shp)).astype(np.float32)
    out = kernel(**iv)
    print("kernel out", out.shape, out.dtype, float(np.abs(out).mean()))
